# revision 31
# baseline (speedup 1.0000x reference)
"""Distributed Trainium2 kernel: LayerNorm -> QKV -> causal MHA -> out-proj.

Sharding (8 cores):
  - LayerNorm + final projection: token-parallel, quarter-blocked: core c
    owns tokens [256c, 256c+256) u [2048+256c, 2048+256c+256) (4096 tokens
    -> 512/core).  Contiguous 256-token blocks keep every DMA descriptor
    >= 2KB (the previous 128-token interleave fragmented the re-load of
    gathered activations into 32K x 256B descriptors).
  - Attention + QKV: head-parallel (16 heads -> 2/core).
  - Comms: 2 pipelined AllGathers of xn^T (bf16, one per batch) after LN so
    QKV starts after the first lands; 2x AllToAll of per-head attention
    output (one per batch half) re-shards to token-parallel for out-proj.

Layout notes:
  - All activations are kept TRANSPOSED ([feature, token]) so every matmul
    contraction runs over the partition axis.  S is computed transposed
    (S^T[j,i] = k_j . q_i); softmax sums come from an appended ones-column
    on V (m=65 matmul); the dh^-0.5 scale rides the exp activation's free
    affine.  Streams are causally trimmed at 128 granularity; the single
    partial diagonal 128x128 tile is zeroed with a triangle mask post-exp.
  - The two heads' K=64 S-matmuls are emitted back-to-back at row-disjoint
    tile_position (0,0)/(64,0) so they co-execute on the PE array.
  - gamma/beta are folded host-side into w_qkv (row scale) and per-feature
    biases (beta @ w_qkv); weights are pre-laid-out host-side as
    [partition, ktile, col] so weight DMAs are contiguous.
  - Matmul inputs are bf16 (4x the fp32 TensorE rate); accumulation fp32.
  - Attention groups are interleaved into the QKV token-chunk loop so
    ScalarE exp work overlaps TensorE QKV matmuls.  Per-chunk AllToAll
    staging (SWDGE) happens as each attention chunk completes; the first
    half's out-projection is interleaved into the final attention chunk so
    only AllToAll-2 + half an out-proj remain in the tail.
"""

import numpy as np
import ml_dtypes

import concourse.bass as bass
import concourse.tile as tile
from concourse import bacc, mybir
from concourse.bass import ds, ts
from concourse.bass_utils import run_bass_kernel_spmd
from concourse.masks import make_identity

B, N, D = 2, 2048, 1024
HEADS, DH = 16, 64
INNER = HEADS * DH          # 1024
NCORES = 8
T = B * N                   # 4096 tokens
TS = T // NCORES            # 512 tokens per core
QTOK = 256                  # tokens per quarter-block (per core per batch)
HPC = HEADS // NCORES       # 2 heads per core
SCALE = float(DH) ** -0.5   # 0.125
EPS = 1e-5

FP = mybir.dt.float32
BF = mybir.dt.bfloat16

KT = D // 128               # 8 contraction tiles of 128 over dim
TCH = T // 512              # 8 token chunks of 512
ICB = N // 512              # 4 i-chunks of 512 per batch
JTB = N // 128              # 16 j-tiles of 128 per batch


def build():
    nc = bacc.Bacc("TRN2", target_bir_lowering=False, debug=False,
                   num_devices=NCORES)

    x_sh = nc.dram_tensor("x_shard", [TS, D], FP, kind="ExternalInput")
    wq_t = nc.dram_tensor("wq", [128, KT, HPC * DH], BF, kind="ExternalInput")
    wk_t = nc.dram_tensor("wk", [128, KT, HPC * DH], BF, kind="ExternalInput")
    wv_t = nc.dram_tensor("wv", [128, KT, HPC * DH], BF, kind="ExternalInput")
    wb_t = nc.dram_tensor("wb", [128, 3], FP, kind="ExternalInput")
    wo_t = nc.dram_tensor("w_out", [128, KT, D], BF, kind="ExternalInput")
    out_sh = nc.dram_tensor("out_shard", [TS, D], FP, kind="ExternalOutput")

    with tile.TileContext(nc) as tc:
        _body(nc, tc, x_sh, wq_t, wk_t, wv_t, wb_t, wo_t, out_sh)

    nc.compile()
    return nc


def _att_thunks(nc, tci, kTt, qT, vhat, tri, outO, a2a_in,
                s_ps, av_ps, espool, smallp):
    """Attention for query chunk tci as a list of PE-ordered thunks.

    Each jp step emits the two heads' S matmuls in a single thunk pair so
    the row-disjoint (tile_position 0 / 64) K=64 matmuls stay adjacent in
    the PE queue and co-execute.  S-matmuls for step jp are emitted before
    the AV-matmuls of step jp-1 so the PE never waits on ACT exp latency.
    The normalized output lands in a per-chunk [64, 2h, 512] tile which is
    shipped to the AllToAll bounce buffer as the final thunk.
    """
    b, ic = tci // ICB, tci % ICB
    q_idx = tci
    njt = 4 * (ic + 1)
    av = [av_ps.tile([128, 512], FP, tag=f"av{h}", name=f"av{h}_{q_idx}")
          for h in range(HPC)]
    oO = outO.tile([64, HPC, 512], BF, tag="oO", name=f"oO_{q_idx}")
    es = {}

    def i0_of(jt):
        m = jt - 4 * ic
        return 128 * m if m > 0 else 0

    def s_pair(jp, u):
        # Both heads' S^T for j-tile jt land in ONE PSUM tile (h0 cols
        # 0-511, h1 cols 512-1023) so a single exp releases the slot and
        # the scheduler cannot split the co-executing pair.
        def run():
            sx = s_ps.tile([128, 1024], FP, tag="sx",
                           name=f"sx_{q_idx}_{jp}_{u}")
            es[(jp, u)] = (sx, None)
            jt = 2 * jp + u
            tq = b * ICB + jt // 4
            jo = 128 * (jt % 4)
            i0 = i0_of(jt)
            for h in range(HPC):
                nc.tensor.matmul(
                    sx[:, ds(512 * h + i0, 512 - i0)],
                    kTt[ds(64 * h, 64), tq, ds(jo, 128)],
                    qT[ds(64 * h, 64), q_idx, ds(i0, 512 - i0)],
                    start=True, stop=True,
                    tile_position=(64 * h, 0))
        return run

    def exp_t(jp, u):
        def run():
            sx, _ = es[(jp, u)]
            e = espool.tile([128, 1024], BF, tag="es",
                            name=f"es_{q_idx}_{jp}_{u}")
            es[(jp, u)] = (sx, e)
            i0 = i0_of(2 * jp + u)
            if i0 == 0:
                nc.scalar.activation(
                    out=e, in_=sx,
                    func=mybir.ActivationFunctionType.Exp, scale=SCALE)
            else:
                nc.scalar.activation(
                    out=e.rearrange("p (h i) -> p h i", h=2)[:, :, i0:],
                    in_=sx.rearrange("p (h i) -> p h i", h=2)[:, :, i0:],
                    func=mybir.ActivationFunctionType.Exp, scale=SCALE)
        return run

    def av_mm(h, jp, u):
        def run():
            _, e = es[(jp, u)]
            jt = 2 * jp + u
            m = jt - 4 * ic
            i0 = i0_of(jt)
            if 0 <= m < 4:
                # zero the strict upper triangle of the diagonal 128x128 tile
                nc.vector.tensor_tensor(
                    out=e[:, ds(512 * h + 128 * m, 128)],
                    in0=e[:, ds(512 * h + 128 * m, 128)],
                    in1=tri,
                    op=mybir.AluOpType.mult)
            nc.tensor.matmul(
                av[h][0:65, ds(i0, 512 - i0)],
                vhat[:, b * JTB + jt, ds(65 * h, 65)],
                e[:, ds(512 * h + i0, 512 - i0)],
                start=(jt == 0), stop=(jt == njt - 1))
        return run

    def norm(h):
        def run():
            # rsum bounce on ScalarE: ACT is idle once the chunk's exps
            # are done, and this keeps the two heads' norm chains from
            # serializing on DVE ahead of the final AllToAll trigger.
            rsum = smallp.tile([1, 512], FP, tag="rsum", name=f"rs{h}_{q_idx}")
            nc.scalar.copy(out=rsum, in_=av[h][64:65, :])
            rec = smallp.tile([1, 512], FP, tag="rec", name=f"rc{h}_{q_idx}")
            nc.vector.reciprocal_approx_fast(out=rec, in_=rsum)
            bc = smallp.tile([64, 512], FP, tag="bc", name=f"bc{h}_{q_idx}")
            nc.gpsimd.partition_broadcast(bc, rec)
            nc.vector.tensor_tensor(
                out=oO[:, h, :],
                in0=av[h][0:64, :], in1=bc,
                op=mybir.AluOpType.mult)
        return run

    def stage(h):
        # ship this chunk's output to the AllToAll bounce buffer: chunk tci
        # = quarter-blocks of dest cores {2*ic, 2*ic+1} in batch half b.
        # Per-head so each head's rows ship as soon as its norm completes.
        def run():
            for cc in range(2):
                nc.gpsimd.dma_start(
                    out=a2a_in[b, 2 * ic + cc][ds(64 * h, 64), :],
                    in_=oO[:, h, ds(QTOK * cc, QTOK)])
        return run

    def s_group(jp):
        return [s_pair(jp, 0), s_pair(jp, 1), exp_t(jp, 0), exp_t(jp, 1)]

    def av_group(jp):
        return [av_mm(0, jp, 0), av_mm(0, jp, 1),
                av_mm(1, jp, 0), av_mm(1, jp, 1)]

    thunks = []
    nps = njt // 2
    thunks.extend(s_group(0))
    for jp in range(1, nps):
        thunks.extend(s_group(jp))
        thunks.extend(av_group(jp - 1))
    # final AV group with each head's normalization emitted as soon as that
    # head's accumulation stops, so the post-attention latency to the A2A
    # staging DMA is one norm chain, not two.
    thunks.extend([av_mm(0, nps - 1, 0), av_mm(0, nps - 1, 1), norm(0),
                   stage(0), av_mm(1, nps - 1, 0), av_mm(1, nps - 1, 1),
                   norm(1), stage(1)])
    return thunks


def _qkv_thunks(nc, tci, xt_u, wq_sb, wk_sb, wv_sb, wb_sb,
                qT, kTt, vhat, qkv_ps, vst, identity):
    """QKV projection for token chunk tci as a list of PE-ordered thunks.

    xt_u is [128, 4 blk, KT, 128]: four gathered 128-token blocks in chunk
    token order; the matmul rhs streams all 4 blocks of one k-tile via a
    strided AP ([128, 4, 128] = 512 columns).
    """
    thunks = []

    def mk_group(w_sb, nm):
        acc = qkv_ps.tile([128, 512], FP, tag="acc", name=f"acc{nm}_{tci}")

        def mm(k):
            def run():
                nc.tensor.matmul(acc, w_sb[:, k, :], xt_u[:, :, k, :],
                                 start=(k == 0), stop=(k == KT - 1))
            return run
        return acc, mm

    for w_sb, bi, dst, nm in ((wq_sb, 0, qT, "q"),
                              (wk_sb, 1, kTt, "k")):
        acc, mm = mk_group(w_sb, nm)
        for k in range(KT):
            thunks.append(mm(k))

        def copy(acc=acc, bi=bi, dst=dst, tci=tci):
            nc.vector.tensor_scalar(
                out=dst[:, tci, :], in0=acc, scalar1=wb_sb[:, ds(bi, 1)],
                scalar2=None, op0=mybir.AluOpType.add)
        thunks.append(copy)

    accv, mmv = mk_group(wv_sb, "v")
    for k in range(KT):
        thunks.append(mmv(k))
    vs = vst.tile([128, 512], BF, tag="vs", name=f"vs_{tci}")

    def vcopy():
        nc.vector.tensor_scalar(
            out=vs, in0=accv, scalar1=wb_sb[:, 2:3], scalar2=None,
            op0=mybir.AluOpType.add)
    thunks.append(vcopy)

    # PE-mode transpose (not DMA xbar: a DMA transpose would serialize
    # against the collectives' DMA traffic and stall the loop)
    vt = qkv_ps.tile([128, 4, 128], BF, tag="acc", name=f"vt_{tci}")

    def vtr(i):
        def run():
            nc.tensor.transpose(vt[:, i, :], vs[:, ds(128 * i, 128)],
                                identity)
        return run
    for i in range(4):
        thunks.append(vtr(i))

    def vstore():
        nc.vector.tensor_copy(out=vhat[:, ds(tci * 4, 4), 0:64],
                              in_=vt[:, :, 0:64])
        nc.vector.tensor_copy(out=vhat[:, ds(tci * 4, 4), 65:129],
                              in_=vt[:, :, 64:128])
    thunks.append(vstore)
    return thunks


def _merge(primary, filler):
    """Interleave filler thunks evenly between primary thunks."""
    out = []
    np_, nf = len(primary), len(filler)
    fi = 0
    for i, p in enumerate(primary):
        out.append(p)
        want = (i + 1) * nf // np_
        while fi < want:
            out.append(filler[fi])
            fi += 1
    out.extend(filler[fi:])
    return out


def _body(nc, tc, x_sh, wq_t, wk_t, wv_t, wb_t, wo_t, out_sh):
    from contextlib import ExitStack
    ctx = ExitStack()
    with ctx:
        const = ctx.enter_context(tc.tile_pool(name="const", bufs=1))
        wpool = ctx.enter_context(tc.tile_pool(name="wpool", bufs=1))
        big = ctx.enter_context(tc.tile_pool(name="big", bufs=1))
        dram = ctx.enter_context(tc.tile_pool(name="dram", bufs=1, space="DRAM"))

        # ---------- constants ----------
        identity = const.tile([128, 128], BF)
        make_identity(nc, identity)

        # 0/1 lower-triangle mask for the one partial 128x128 diagonal tile
        # of S^T: keep [jj, ii] iff ii - jj >= 0.
        tri = const.tile([128, 128], BF)
        nc.gpsimd.memset(tri, 1.0)
        nc.gpsimd.affine_select(
            out=tri, in_=tri,
            compare_op=mybir.AluOpType.is_ge, fill=0.0,
            base=0, pattern=[[1, 128]], channel_multiplier=-1,
        )

        eps_t = const.tile([128, 1], FP)
        nc.vector.memset(eps_t, EPS)

        wb_sb = const.tile([128, 3], FP)

        # ---------- comm bounce buffers ----------
        # ag layout is [p, s, k, t] with s = 2q+u the four 128-token strips:
        # stores are 2KB-contiguous per partition and each chunk's re-load is
        # two fully-contiguous 4KB-per-partition DMAs.
        ag_in = dram.tile([128, 4, KT, 128], BF)
        ag_out = dram.tile([NCORES, 128, 4, KT, 128], BF,
                           addr_space="Shared")
        a2a_in = dram.tile([2, NCORES, 128, QTOK], BF)
        a2a_out = [dram.tile([NCORES, 128, QTOK], BF,
                             name=f"a2a_out{hh}") for hh in range(2)]
        warm_in = dram.tile([128, 8], BF)
        warm_out = dram.tile([NCORES, 128, 8], BF, addr_space="Shared")

        wq_sb = wpool.tile([128, KT, HPC * DH], BF)
        wk_sb = wpool.tile([128, KT, HPC * DH], BF)
        wv_sb = wpool.tile([128, KT, HPC * DH], BF)
        wo_sb = wpool.tile([128, KT, D], BF)

        # weight loads ride SWDGE so the two HWDGE rings stay free for the
        # latency-critical x-load -> LN -> stage -> gather chain.
        nc.gpsimd.dma_start(out=wq_sb, in_=wq_t.ap())
        nc.gpsimd.dma_start(out=wk_sb, in_=wk_t.ap())
        nc.gpsimd.dma_start(out=wv_sb, in_=wv_t.ap())
        nc.gpsimd.dma_start(out=wb_sb, in_=wb_t.ap())

        # ---------- phase A: per-strip LayerNorm + transpose + AllGather ---
        x_view = x_sh.ap().rearrange("(q u p) d -> q u p d", q=2, p=128)
        with tc.tile_pool(name="lnp", bufs=2) as lnp, \
             tc.tile_pool(name="lns", bufs=2) as lns, \
             tc.tile_pool(name="tstage", bufs=2) as tstage:
            x_tiles = {}
            for q in range(2):
                for u in range(2):
                    x_t = lnp.tile([128, D], FP, tag=f"x{q}{u}",
                                   name=f"x_{q}_{u}", bufs=1)
                    nc.sync.dma_start(out=x_t, in_=x_view[q, u])
                    x_tiles[(q, u)] = x_t
            for q in range(2):
                for u in range(2):
                    x_t = x_tiles[(q, u)]
                    stats = lns.tile([128, 2, 6], FP, tag="stats")
                    xg = x_t.rearrange("p (s f) -> p s f", f=512)
                    for s in range(2):
                        nc.vector.bn_stats(out=stats[:, s, :], in_=xg[:, s, :])
                    mv = lns.tile([128, 2], FP, tag="mv")
                    nc.vector.bn_aggr(out=mv, in_=stats)
                    rstd = lns.tile([128, 1], FP, tag="rstd")
                    nc.scalar.activation(
                        out=rstd, in_=mv[:, 1:2],
                        func=mybir.ActivationFunctionType.Sqrt,
                        bias=eps_t, scale=1.0)
                    nc.vector.reciprocal(out=rstd, in_=rstd)
                    xn_bf = lnp.tile([128, D], BF, tag="xnbf")
                    nc.vector.tensor_scalar(
                        out=xn_bf, in0=x_t, scalar1=mv[:, 0:1], scalar2=rstd,
                        op0=mybir.AluOpType.subtract,
                        op1=mybir.AluOpType.mult)
                    xnT_s = tstage.tile([128, KT, 128], BF, tag="xnT")
                    nc.scalar.dma_start_transpose(out=xnT_s, in_=xn_bf)
                    nc.sync.dma_start(out=ag_in[:, 2 * q + u], in_=xnT_s)
            nc.gpsimd.collective_compute(
                "AllGather", mybir.AluOpType.bypass,
                replica_groups=[list(range(NCORES))],
                ins=[ag_in.opt()], outs=[ag_out.opt()])

        # ---------- phase B+C: QKV + interleaved attention + A2A/out-proj --
        qT = big.tile([128, TCH, 512], BF)   # rows: [h0 64 | h1 64]
        kTt = big.tile([128, TCH, 512], BF)
        vhat = big.tile([128, JTB * B, 130], BF)  # col 64/129 = 1
        nc.gpsimd.memset(vhat[:, :, 64:65], 1.0)
        nc.gpsimd.memset(vhat[:, :, 129:130], 1.0)
        a2a_sb = [big.tile([128, NCORES, QTOK], BF, name=f"a2a_sb{hh}")
                  for hh in range(2)]
        out_view = out_sh.ap().rearrange("(t p) e -> p t e", p=128)

        def op_thunks(hh):
            # output projection for this core's batch-hh quarter-block
            thunks = []
            for rr in range(2):
                tt = 2 * hh + rr
                for ec in range(D // 512):
                    po = qkv_ps.tile([128, 512], FP, tag="acc",
                                     name=f"po_{tt}_{ec}")

                    def mms(po=po, hh=hh, rr=rr, ec=ec):
                        for ct in range(NCORES):
                            nc.tensor.matmul(
                                po, a2a_sb[hh][:, ct, ds(128 * rr, 128)],
                                wo_sb[:, ct, ds(512 * ec, 512)],
                                start=(ct == 0), stop=(ct == NCORES - 1))
                    thunks.append(mms)

                    def store(po=po, tt=tt, ec=ec):
                        ost = ostp.tile([128, 512], FP, tag="ost",
                                        name=f"ost_{tt}_{ec}")
                        nc.vector.tensor_copy(out=ost, in_=po)
                        nc.sync.dma_start(
                            out=out_view[:, tt, ds(512 * ec, 512)], in_=ost)
                    thunks.append(store)
            return thunks

        def op_load(hh):
            # per-source-block loads alternating both HWDGE rings: the OP
            # matmuls consume blocks in ct order, so the first matmul issues
            # after ~64KB lands instead of the full 512KB.
            for ct in range(NCORES):
                eng = nc.sync if ct % 2 == 0 else nc.scalar
                eng.dma_start(out=a2a_sb[hh][:, ct, :],
                              in_=a2a_out[hh][ct])

        with tc.tile_pool(name="xstream", bufs=2) as xstream, \
             tc.tile_pool(name="qkv_ps", bufs=2, space="PSUM") as qkv_ps, \
             tc.tile_pool(name="s_ps", bufs=2, space="PSUM") as s_ps, \
             tc.tile_pool(name="av_ps", bufs=1, space="PSUM") as av_ps, \
             tc.tile_pool(name="espool", bufs=6) as espool, \
             tc.tile_pool(name="smallp", bufs=4) as smallp, \
             tc.tile_pool(name="ostp", bufs=3) as ostp, \
             tc.tile_pool(name="outO", bufs=2) as outO, \
             tc.tile_pool(name="vstage", bufs=2) as vst:

            def xt_load(tci):
                # chunk tci = gathered batch-q strips of cores {2i, 2i+1}:
                # one fully-contiguous 4KB-per-partition DMA per source core,
                # one source core per HWDGE ring so the two halves land in
                # parallel (a single ring serializes them: ~7us vs ~3.5us).
                q, i = tci // ICB, tci % ICB
                xtu = xstream.tile([128, 4, KT, 128], BF, tag=f"xt{tci % 2}",
                                   name=f"xt_{tci}")
                for cc, eng in ((0, nc.sync), (1, nc.scalar)):
                    eng.dma_start(
                        out=xtu[:, ds(2 * cc, 2)],
                        in_=ag_out[2 * i + cc][:, ds(2 * q, 2)])
                return xtu

            xts = {0: xt_load(0), 1: xt_load(1)}
            pending_att = None
            for tci in range(TCH):
                if tci + 2 < TCH:
                    xts[tci + 2] = xt_load(tci + 2)
                if tci == 0:
                    nc.gpsimd.dma_start(out=wo_sb[:, 0:4, :],
                                        in_=wo_t.ap()[:, 0:4, :])
                if tci == 2:
                    nc.gpsimd.dma_start(out=wo_sb[:, 4:8, :],
                                        in_=wo_t.ap()[:, 4:8, :])
                qkv = _qkv_thunks(nc, tci, xts.pop(tci),
                                  wq_sb, wk_sb, wv_sb, wb_sb,
                                  qT, kTt, vhat, qkv_ps, vst, identity)
                seq = qkv if pending_att is None else _merge(pending_att, qkv)
                for thunk in seq:
                    thunk()
                if tci == 4:
                    # batch-0 attention (chunks 0-3) fully emitted + staged
                    nc.gpsimd.collective_compute(
                        "AllToAll", mybir.AluOpType.bypass,
                        replica_groups=[list(range(NCORES))],
                        ins=[a2a_in[0].opt()], outs=[a2a_out[0].opt()])
                if tci == 5:
                    op_load(0)
                pending_att = _att_thunks(nc, tci, kTt, qT, vhat, tri,
                                          outO, a2a_in, s_ps, av_ps,
                                          espool, smallp)
            # final attention chunk, then A2A-2; the first-half out-proj
            # (A2A-1 landed chunks ago) fills the PE during A2A-2 flight.
            for thunk in pending_att:
                thunk()
            nc.gpsimd.collective_compute(
                "AllToAll", mybir.AluOpType.bypass,
                replica_groups=[list(range(NCORES))],
                ins=[a2a_in[1].opt()], outs=[a2a_out[1].opt()])
            op_load(1)
            for thunk in op_thunks(0):
                thunk()
            for thunk in op_thunks(1):
                thunk()


_NC = None
LAST_EXEC_TIME_NS = None


def _get_nc():
    global _NC
    if _NC is None:
        _NC = build()
    return _NC


def make_in_maps(x, gamma, beta, w_qkv, w_out):
    bf = ml_dtypes.bfloat16
    x = np.ascontiguousarray(np.asarray(x, dtype=np.float32)).reshape(T, D)
    gamma = np.asarray(gamma, dtype=np.float32)
    beta = np.asarray(beta, dtype=np.float32)
    w_qkv = np.asarray(w_qkv, dtype=np.float32)
    qkv_bias = beta @ w_qkv                      # [3*INNER]
    w_qkv = (w_qkv * gamma[:, None]).astype(bf)  # fold gamma into weights
    w_out = np.asarray(w_out, dtype=np.float32).astype(bf)
    # weights pre-laid-out [partition, ktile, col] for contiguous DMA
    wo_pkc = np.ascontiguousarray(
        w_out.reshape(KT, 128, D).transpose(1, 0, 2))
    xs = x.reshape(2, NCORES, QTOK, D)
    in_maps = []
    for c in range(NCORES):
        cols = slice(128 * c, 128 * c + 128)
        wq = w_qkv[:, cols].reshape(KT, 128, 128).transpose(1, 0, 2)
        wk = w_qkv[:, INNER:][:, cols].reshape(KT, 128, 128).transpose(1, 0, 2)
        wv = (w_qkv[:, 2 * INNER:][:, cols]
              .reshape(KT, 128, 128).transpose(1, 0, 2))
        wb = np.stack([qkv_bias[cols], qkv_bias[INNER:][cols],
                       qkv_bias[2 * INNER:][cols]], axis=1)
        in_maps.append({
            "x_shard": np.ascontiguousarray(xs[:, c].reshape(TS, D)),
            "wq": np.ascontiguousarray(wq),
            "wk": np.ascontiguousarray(wk),
            "wv": np.ascontiguousarray(wv),
            "wb": np.ascontiguousarray(wb.astype(np.float32)),
            "w_out": wo_pkc,
        })
    return in_maps


def assemble_out(results):
    out = np.zeros((2, NCORES, QTOK, D), dtype=np.float32)
    for c in range(NCORES):
        out[:, c] = results[c]["out_shard"].reshape(2, QTOK, D)
    return out.reshape(B, N, D)


def kernel(x, mask, gamma, beta, w_qkv, w_out):
    global LAST_EXEC_TIME_NS
    nc = _get_nc()
    in_maps = make_in_maps(x, gamma, beta, w_qkv, w_out)
    res = run_bass_kernel_spmd(nc, in_maps, core_ids=list(range(NCORES)))
    LAST_EXEC_TIME_NS = res.exec_time_ns
    return assemble_out(res.results).astype(np.float32)
